# revision 17
# baseline (speedup 1.0000x reference)
"""Trainium2 Bass kernel for CLSProcess: diagonal linear recurrence
state_t = y_t * state_{t-1} + x_t * z_t over [B=8, T=4096, units=1024].

Sharding: batch across the 8 cores (one batch element per core).

v3 design (~2x over the v1 baseline):
  - all matmuls bf16 (f32r lowers to 4-pass fp32 "fp32_mode=HIGH" on this
    toolchain: ~755ns per 512-col matmul vs ~213ns bf16). z arrives in
    SBUF already in bf16 via gpsimd SWDGE cast-DMAs (f32->bf16 in
    flight), one DMA per 4 blocks with a "(a b) c -> b (a c)" rearrange
    so partition p holds rows {g*512+p, +128, +256, +384}.
  - x and y never need a transpose: a [128, 8] sideband DMA per group
    (columns 0:2 rearranged) gives per-block x/y columns, and one
    prologue SWDGE gather pulls the whole y row [1, 4096] (block-start
    positions re-zeroed by a strided memset for the scan reset).
  - per block, the decay matrix M[t,s] = prod y is built by a DVE
    tensor_tensor_scan over the identity (batched 4 blocks/scan), then
    x is folded into the bf16 weight download (activation Copy with
    scale=x column): lhsT = x_s*M[t,s].
  - carry term: po[t,:] += p_t * prev[127,:] via a rank-1 bf16 matmul
    with sel[s,t] = I[s==127]*p_t (full K=128: base-64 K=64 tiles
    silently corrupt the accumulation group).
  - output written bf16 (halves output traffic; host upconverts),
    single PSUM drain per block split DVE/Act, two blocks batched per
    output DMA (SP queue) via the same DRAM rearrange.
  - DMA traffic/core: 16.8 MB in + 8.4 MB out = 25.2 MB -> ~70us floor
    at 16 DMA engines x 22.5 B/ns.
"""

import numpy as np

import concourse.bacc as bacc
import concourse.bass as bass
import concourse.mybir as mybir
import concourse.tile as tile
from concourse.bass_utils import run_bass_kernel_spmd

B = 8
T = 4096
F = 1026
U = 1024
L = 128
G = 4  # blocks per group (one scan / one z cast-DMA per group)
OB = 2  # blocks per output DMA
f32 = mybir.dt.float32
f32r = mybir.dt.float32r
bf16 = mybir.dt.bfloat16
Copy = mybir.ActivationFunctionType.Copy


def build_nc(t_total: int = T) -> bass.Bass:
    nb = t_total // L
    ng = nb // G
    nc = bacc.Bacc()
    inp = nc.dram_tensor("inp", [t_total, F], f32, kind="ExternalInput")
    out = nc.dram_tensor("out", [t_total, U], bf16, kind="ExternalOutput")
    ident4_d = nc.inline_tensor(
        np.tile(np.eye(L, dtype=np.float32), (1, G)), name="ident4"
    )
    e127c_np = np.zeros((L, 1), dtype=np.float32)
    e127c_np[L - 1, 0] = 1.0
    e127c_d = nc.inline_tensor(e127c_np, name="e127c")

    with tile.TileContext(nc) as tc:
        with (
            tc.tile_pool(name="const", bufs=1) as constp,
            tc.tile_pool(name="yrow", bufs=1) as yrowp,
            tc.tile_pool(name="zpool", bufs=8) as zpool,
            tc.tile_pool(name="mpool", bufs=4) as mpool,
            tc.tile_pool(name="mscpool", bufs=14) as mscpool,
            tc.tile_pool(name="rowpool", bufs=14) as rowpool,
            tc.tile_pool(name="bcpool", bufs=8) as bcpool,
            tc.tile_pool(name="pbpool", bufs=14) as pbpool,
            tc.tile_pool(name="selpool", bufs=14) as selpool,
            tc.tile_pool(name="otbpool", bufs=3) as otbpool,
            tc.tile_pool(name="ps_out", bufs=4, space="PSUM") as ps_out_pool,
        ):
            ident4 = constp.tile([L, G * L], f32, tag="ident4")
            nc.sync.dma_start(ident4[:], ident4_d[:, :])
            e127c = constp.tile([L, 1], f32, tag="e127c")
            nc.sync.dma_start(e127c[:], e127c_d[:, :])

            zts = {}
            ybcs = {}

            def dispatch_z(g: int):
                r0 = g * G * L
                # z for 4 blocks, cast f32->bf16 in the DMA (SWDGE):
                # partition p <- rows {r0+p, r0+128+p, r0+256+p, r0+384+p}
                zt = zpool.tile([L, G * U], bf16, tag="zt")
                nc.gpsimd.dma_start(
                    zt[:],
                    inp[r0 : r0 + G * L, 2:F].rearrange("(a b) c -> b a c", a=G),
                )
                zts[g] = zt

            # prologue: first z group, then the sideband rows (y gather with
            # block-start re-zeroing; x/y0 columns for all 32 blocks in one
            # SWDGE DMA), then the remaining z groups + y broadcasts.
            dispatch_z(0)
            yz3 = yrowp.tile([1, nb, L], f32, tag="yz3")
            nc.gpsimd.dma_start(yz3[0:1, :, :], inp[:, 1:2].rearrange("a b -> b a"))
            xyall = constp.tile([L, nb * 2], f32, tag="xyall")
            nc.gpsimd.dma_start(
                xyall[:], inp[:, 0:2].rearrange("(a b) c -> b a c", a=nb)
            )
            nc.gpsimd.memset(yz3[0:1, :, 0:1], 0.0)

            def broadcast_y(g: int):
                ybc = bcpool.tile([L, G * L], f32, tag="ybc")
                nc.gpsimd.partition_broadcast(ybc[:], yz3[0:1, G * g : G * (g + 1), :])
                ybcs[g] = ybc

            broadcast_y(0)
            for g in range(1, ng):
                dispatch_z(g)
                broadcast_y(g)

            mt4s = {}
            mscs = {}
            sels = {}
            pos = {}
            otbs = {}

            def prep(g: int):
                # everything for group g that does not touch block outputs:
                # the M scan, bf16 weights, and the carry selector tiles
                mt4 = mpool.tile([L, G * L], f32r, tag="mt4")
                nc.vector.tensor_tensor_scan(
                    mt4[:],
                    ybcs.pop(g)[:],
                    ident4[:],
                    0.0,
                    mybir.AluOpType.mult,
                    mybir.AluOpType.add,
                )
                mt4s[g] = mt4
                for j in range(G):
                    k = g * G + j
                    mtk = mt4[:, L * j : L * j + L]
                    xcol = xyall[:, 2 * k : 2 * k + 1]
                    y0 = xyall[0:1, 2 * k + 1 : 2 * k + 2]
                    # bf16 weights: msc[s, t] = x_s * M[t, s]
                    msc = mscpool.tile([L, L], bf16, tag="msc")
                    nc.scalar.activation(msc[:], mtk, Copy, scale=xcol)
                    mscs[k] = msc
                    if k > 0:
                        # p_t = prod_{r=block_start..t} y_r = y_0 * mt[0, t]
                        prow = rowpool.tile([1, L], f32, tag="prow")
                        nc.vector.tensor_scalar_mul(prow[:], mtk[0:1, :], y0)
                        # sel[s, t] = I[s==127] * p_t
                        pb = pbpool.tile([L, L], f32, tag="pb")
                        nc.gpsimd.partition_broadcast(pb[:], prow[0:1, :])
                        sel = selpool.tile([L, L], bf16, tag="sel")
                        nc.scalar.activation(sel[:], pb[:], Copy, scale=e127c[:])
                        sels[k] = sel

            def mains(k: int):
                g, j = divmod(k, G)
                po = ps_out_pool.tile([L, U], f32, tag="po")
                msc = mscs.pop(k)
                zt = zts[g]
                for jj in (0, 512):
                    nc.tensor.matmul(
                        po[:, jj : jj + 512],
                        msc[:],
                        zt[:, j * U + jj : j * U + jj + 512],
                        start=True,
                        stop=(k == 0),
                    )
                pos[k] = po

            def back(k: int):
                # carry accumulation + drain + output for block k
                po = pos.pop(k)
                if k > 0:
                    # po[t, :] += p_t * prev[127, :]
                    sel = sels.pop(k)
                    pt, pc = otbs[k - 1]
                    for jj in (0, 512):
                        nc.tensor.matmul(
                            po[:, jj : jj + 512],
                            sel[:],
                            pt[:, pc + jj : pc + jj + 512],
                            start=False,
                            stop=True,
                        )
                    otbs.pop(k - 1, None)
                # single bf16 drain, split DVE/Act; OB blocks share one
                # otb tile -> one batched output DMA
                h = k % OB
                if h == 0:
                    otb = otbpool.tile([L, OB * U], bf16, tag="otb")
                    otbs["cur"] = otb
                otb = otbs["cur"]
                c0 = h * U
                nc.vector.tensor_copy(otb[:, c0 : c0 + 512], po[:, 0:512])
                nc.scalar.copy(otb[:, c0 + 512 : c0 + U], po[:, 512:U])
                otbs[k] = (otb, c0)
                if h == OB - 1:
                    r0b = (k - OB + 1) * L
                    nc.sync.dma_start(
                        out[r0b : r0b + OB * L, :].rearrange(
                            "(a b) c -> b a c", a=OB
                        ),
                        otb[:],
                    )

            # software pipeline: prep runs 2 groups ahead; PE executes block
            # k's mains while block k-1's carry waits on the drain
            prep(0)
            prep(1)
            prep(2)
            mains(0)
            for k in range(1, nb):
                if k % G == 0 and k // G + 2 <= ng - 1:
                    prep(k // G + 2)
                mains(k)
                back(k - 1)
            back(nb - 1)
    nc.finalize()
    return nc


_NC = None


def _get_nc() -> bass.Bass:
    global _NC
    if _NC is None:
        _NC = build_nc()
    return _NC


def kernel(**inputs: np.ndarray) -> np.ndarray:
    x = np.ascontiguousarray(inputs["inputs"], dtype=np.float32)
    assert x.shape == (B, T, F), x.shape
    nc = _get_nc()
    in_maps = [{"inp": x[c]} for c in range(B)]
    res = run_bass_kernel_spmd(nc, in_maps, core_ids=list(range(B)))
    return np.stack(
        [np.asarray(res.results[c]["out"]).astype(np.float32) for c in range(B)],
        axis=0,
    )


# revision 18
# speedup vs baseline: 1.0091x; 1.0091x over previous
"""Trainium2 Bass kernel for CLSProcess: diagonal linear recurrence
state_t = y_t * state_{t-1} + x_t * z_t over [B=8, T=4096, units=1024].

Sharding: batch across the 8 cores (one batch element per core).

v3 design (~2x over the v1 baseline):
  - all matmuls bf16 (f32r lowers to 4-pass fp32 "fp32_mode=HIGH" on this
    toolchain: ~755ns per 512-col matmul vs ~213ns bf16). z arrives in
    SBUF already in bf16 via gpsimd SWDGE cast-DMAs (f32->bf16 in
    flight), one DMA per 4 blocks with a "(a b) c -> b (a c)" rearrange
    so partition p holds rows {g*512+p, +128, +256, +384}.
  - x and y never need a transpose: a [128, 8] sideband DMA per group
    (columns 0:2 rearranged) gives per-block x/y columns, and one
    prologue SWDGE gather pulls the whole y row [1, 4096] (block-start
    positions re-zeroed by a strided memset for the scan reset).
  - per block, the decay matrix M[t,s] = prod y is built by a DVE
    tensor_tensor_scan over the identity (batched 4 blocks/scan), then
    x is folded into the bf16 weight download (activation Copy with
    scale=x column): lhsT = x_s*M[t,s].
  - carry term: po[t,:] += p_t * prev[127,:] via a rank-1 bf16 matmul
    with sel[s,t] = I[s==127]*p_t (full K=128: base-64 K=64 tiles
    silently corrupt the accumulation group).
  - output written bf16 (halves output traffic; host upconverts),
    single PSUM drain per block split DVE/Act, two blocks batched per
    output DMA (SP queue) via the same DRAM rearrange.
  - DMA traffic/core: 16.8 MB in + 8.4 MB out = 25.2 MB -> ~70us floor
    at 16 DMA engines x 22.5 B/ns.
"""

import numpy as np

import concourse.bacc as bacc
import concourse.bass as bass
import concourse.mybir as mybir
import concourse.tile as tile
from concourse.bass_utils import run_bass_kernel_spmd

B = 8
T = 4096
F = 1026
U = 1024
L = 128
G = 4  # blocks per group (one scan / one z cast-DMA per group)
OB = 2  # blocks per output DMA
f32 = mybir.dt.float32
f32r = mybir.dt.float32r
bf16 = mybir.dt.bfloat16
Copy = mybir.ActivationFunctionType.Copy


def build_nc(t_total: int = T) -> bass.Bass:
    nb = t_total // L
    ng = nb // G
    nc = bacc.Bacc()
    inp = nc.dram_tensor("inp", [t_total, F], f32, kind="ExternalInput")
    out = nc.dram_tensor("out", [t_total, U], bf16, kind="ExternalOutput")
    ident4_d = nc.inline_tensor(
        np.tile(np.eye(L, dtype=np.float32), (1, G)), name="ident4"
    )
    e127c_np = np.zeros((L, 1), dtype=np.float32)
    e127c_np[L - 1, 0] = 1.0
    e127c_d = nc.inline_tensor(e127c_np, name="e127c")

    with tile.TileContext(nc) as tc:
        with (
            tc.tile_pool(name="const", bufs=1) as constp,
            tc.tile_pool(name="yrow", bufs=1) as yrowp,
            tc.tile_pool(name="zpool", bufs=8) as zpool,
            tc.tile_pool(name="mpool", bufs=3) as mpool,
            tc.tile_pool(name="mscpool", bufs=14) as mscpool,
            tc.tile_pool(name="rowpool", bufs=14) as rowpool,
            tc.tile_pool(name="bcpool", bufs=8) as bcpool,
            tc.tile_pool(name="pbpool", bufs=14) as pbpool,
            tc.tile_pool(name="selpool", bufs=14) as selpool,
            tc.tile_pool(name="otbpool", bufs=3) as otbpool,
            tc.tile_pool(name="ps_out", bufs=4, space="PSUM") as ps_out_pool,
        ):
            ident4 = constp.tile([L, G * L], f32, tag="ident4")
            nc.sync.dma_start(ident4[:], ident4_d[:, :])
            e127c = constp.tile([L, 1], f32, tag="e127c")
            nc.sync.dma_start(e127c[:], e127c_d[:, :])

            zts = {}
            ybcs = {}

            def dispatch_z(g: int):
                r0 = g * G * L
                # z for 4 blocks, cast f32->bf16 in the DMA (SWDGE):
                # partition p <- rows {r0+p, r0+128+p, r0+256+p, r0+384+p}
                zt = zpool.tile([L, G * U], bf16, tag="zt")
                nc.gpsimd.dma_start(
                    zt[:],
                    inp[r0 : r0 + G * L, 2:F].rearrange("(a b) c -> b a c", a=G),
                )
                zts[g] = zt

            # prologue: first z group, then the sideband rows (y gather with
            # block-start re-zeroing; x/y0 columns for all 32 blocks in one
            # SWDGE DMA), then the remaining z groups + y broadcasts.
            dispatch_z(0)
            yz3 = yrowp.tile([1, nb, L], f32, tag="yz3")
            nc.gpsimd.dma_start(yz3[0:1, :, :], inp[:, 1:2].rearrange("a b -> b a"))
            xyall = constp.tile([L, nb * 2], f32, tag="xyall")
            nc.gpsimd.dma_start(
                xyall[:], inp[:, 0:2].rearrange("(a b) c -> b a c", a=nb)
            )
            nc.gpsimd.memset(yz3[0:1, :, 0:1], 0.0)

            def broadcast_y(g: int):
                ybc = bcpool.tile([L, G * L], f32, tag="ybc")
                nc.gpsimd.partition_broadcast(ybc[:], yz3[0:1, G * g : G * (g + 1), :])
                ybcs[g] = ybc

            broadcast_y(0)
            for g in range(1, ng):
                dispatch_z(g)
                broadcast_y(g)

            mt4s = {}
            mscs = {}
            sels = {}
            pos = {}
            otbs = {}

            def scan_group(g: int):
                mt4 = mpool.tile([L, G * L], f32r, tag="mt4")
                nc.vector.tensor_tensor_scan(
                    mt4[:],
                    ybcs.pop(g)[:],
                    ident4[:],
                    0.0,
                    mybir.AluOpType.mult,
                    mybir.AluOpType.add,
                )
                mt4s[g] = mt4

            def prep_block(k: int):
                # weights + carry selector for block k (runs well ahead of PE)
                g, j = divmod(k, G)
                mt4 = mt4s[g]
                mtk = mt4[:, L * j : L * j + L]
                xcol = xyall[:, 2 * k : 2 * k + 1]
                y0 = xyall[0:1, 2 * k + 1 : 2 * k + 2]
                # bf16 weights: msc[s, t] = x_s * M[t, s]
                msc = mscpool.tile([L, L], bf16, tag="msc")
                nc.scalar.activation(msc[:], mtk, Copy, scale=xcol)
                mscs[k] = msc
                if k > 0:
                    # p_t = prod_{r=block_start..t} y_r = y_0 * mt[0, t]
                    prow = rowpool.tile([1, L], f32, tag="prow")
                    nc.vector.tensor_scalar_mul(prow[:], mtk[0:1, :], y0)
                    # sel[s, t] = I[s==127] * p_t
                    pb = pbpool.tile([L, L], f32, tag="pb")
                    nc.gpsimd.partition_broadcast(pb[:], prow[0:1, :])
                    sel = selpool.tile([L, L], bf16, tag="sel")
                    nc.scalar.activation(sel[:], pb[:], Copy, scale=e127c[:])
                    sels[k] = sel

            def mains(k: int):
                g, j = divmod(k, G)
                po = ps_out_pool.tile([L, U], f32, tag="po")
                msc = mscs.pop(k)
                zt = zts[g]
                for jj in (0, 512):
                    nc.tensor.matmul(
                        po[:, jj : jj + 512],
                        msc[:],
                        zt[:, j * U + jj : j * U + jj + 512],
                        start=True,
                        stop=(k == 0),
                    )
                pos[k] = po

            def back(k: int):
                # carry accumulation + drain + output for block k
                po = pos.pop(k)
                if k > 0:
                    # po[t, :] += p_t * prev[127, :]
                    sel = sels.pop(k)
                    pt, pc = otbs[k - 1]
                    for jj in (0, 512):
                        nc.tensor.matmul(
                            po[:, jj : jj + 512],
                            sel[:],
                            pt[:, pc + jj : pc + jj + 512],
                            start=False,
                            stop=True,
                        )
                    otbs.pop(k - 1, None)
                # single bf16 drain, split DVE/Act; OB blocks share one
                # otb tile -> one batched output DMA
                h = k % OB
                if h == 0:
                    otb = otbpool.tile([L, OB * U], bf16, tag="otb")
                    otbs["cur"] = otb
                otb = otbs["cur"]
                c0 = h * U
                nc.vector.tensor_copy(otb[:, c0 : c0 + 512], po[:, 0:512])
                nc.scalar.copy(otb[:, c0 + 512 : c0 + U], po[:, 512:U])
                otbs[k] = (otb, c0)
                if h == OB - 1:
                    r0b = (k - OB + 1) * L
                    nc.sync.dma_start(
                        out[r0b : r0b + OB * L, :].rearrange(
                            "(a b) c -> b a c", a=OB
                        ),
                        otb[:],
                    )

            # software pipeline: scans 2 groups ahead, per-block prep 8
            # blocks ahead, carries 2 blocks behind mains so the PE never
            # waits on the drain chain
            PREP_AHEAD = 8
            scan_group(0)
            scan_group(1)
            for kk in range(PREP_AHEAD):
                prep_block(kk)
            for k in range(nb):
                g = k // G
                if k % G == 0 and g + 2 <= ng - 1:
                    scan_group(g + 2)
                if k + PREP_AHEAD < nb:
                    prep_block(k + PREP_AHEAD)
                mains(k)
                if k >= 2:
                    back(k - 2)
            back(nb - 2)
            back(nb - 1)
    nc.finalize()
    return nc


_NC = None


def _get_nc() -> bass.Bass:
    global _NC
    if _NC is None:
        _NC = build_nc()
    return _NC


def kernel(**inputs: np.ndarray) -> np.ndarray:
    x = np.ascontiguousarray(inputs["inputs"], dtype=np.float32)
    assert x.shape == (B, T, F), x.shape
    nc = _get_nc()
    in_maps = [{"inp": x[c]} for c in range(B)]
    res = run_bass_kernel_spmd(nc, in_maps, core_ids=list(range(B)))
    return np.stack(
        [np.asarray(res.results[c]["out"]).astype(np.float32) for c in range(B)],
        axis=0,
    )
